# revision 5
# baseline (speedup 1.0000x reference)
"""Distributed multi-head attention (RoPE + SDPA + out-proj) for one TRN2 chip.

Sharding: 8 cores = 4 batches x 2 query-halves (zero collectives).  v2
restructure vs baseline:
  - projections for o-tile ct (a head pair) interleave with attention of the
    same pair, so ACT exp work overlaps PE projection work
  - scores: one N=1024 bf16-PSUM matmul per (head, kt) (no accumulation, so
    bf16 PSUM out is exact-enough); head pairs sit at partitions 0-63/64-127,
    emitted adjacently -> PE row-group packing runs them concurrently
  - exp: one ACT instruction per (head, kt) over [128, 1024]
  - softmax-denominator broadcast copy moved ACT -> DVE
  - xq kept as input (per-core query-half offset cannot be in SPMD code)
"""

from contextlib import ExitStack, nullcontext

import ml_dtypes
import numpy as np

import concourse.bass as bass
import concourse.tile as tile
from concourse import bacc, bass_utils, mybir

B, S, D, H = 4, 2048, 1024, 16
DH = D // H
SQ = S // 2          # queries per core
NCORES = 8
BF = mybir.dt.bfloat16
F32 = mybir.dt.float32
BF_NP = ml_dtypes.bfloat16

KT_D = D // 128      # 8  d-tiles (contraction for projections)
KT_S = S // 128      # 16 seq k-tiles
KT_SQ = SQ // 128    # 8  out t-tiles
NQ = SQ // 512       # 2  q chunks (512-wide, fp32-psum granularity)
NS = S // 512        # 4  seq chunks
NO = D // 512        # 2  o chunks
NCT = KT_D           # 8  head-pair tiles


def _build(reps=1, pad_scores=True):
    nc = bacc.Bacc("TRN2", target_bir_lowering=False, debug=False,
                   num_devices=NCORES)

    xT = nc.dram_tensor("xT", [D, S], BF, kind="ExternalInput").ap()
    xqT = nc.dram_tensor("xqT", [D, SQ], BF, kind="ExternalInput").ap()
    wqT = nc.dram_tensor("wqT", [D, D], BF, kind="ExternalInput").ap()
    wkT = nc.dram_tensor("wkT", [D, D], BF, kind="ExternalInput").ap()
    wvT = nc.dram_tensor("wvT", [D, D], BF, kind="ExternalInput").ap()
    woT = nc.dram_tensor("woT", [D, D], BF, kind="ExternalInput").ap()
    cosq = nc.dram_tensor("cosq", [128, SQ], BF, kind="ExternalInput").ap()
    sinrq = nc.dram_tensor("sinrq", [128, SQ], BF, kind="ExternalInput").ap()
    cosk = nc.dram_tensor("cosk", [128, S], BF, kind="ExternalInput").ap()
    sinrk = nc.dram_tensor("sinrk", [128, S], BF, kind="ExternalInput").ap()
    out = nc.dram_tensor("out", [SQ, D], F32, kind="ExternalOutput").ap()

    with tile.TileContext(nc) as tc, \
         (tc.For_i(0, reps) if reps > 1 else nullcontext()), \
         ExitStack() as ctx:
        qr_pool = ctx.enter_context(tc.tile_pool(name="qr", bufs=2))
        kr_pool = ctx.enter_context(tc.tile_pool(name="kr", bufs=2))
        v_pool = ctx.enter_context(tc.tile_pool(name="v", bufs=KT_S))
        ctxT_pool = ctx.enter_context(tc.tile_pool(name="ctxT", bufs=KT_D))
        x_pool = ctx.enter_context(tc.tile_pool(name="x", bufs=1))
        xq_pool = ctx.enter_context(tc.tile_pool(name="xq", bufs=1))
        wv_pool = ctx.enter_context(tc.tile_pool(name="wv", bufs=1))
        wkq_pool = ctx.enter_context(tc.tile_pool(name="wkq", bufs=4))
        raw_pool = ctx.enter_context(tc.tile_pool(name="raw", bufs=2))
        rot_pool = ctx.enter_context(tc.tile_pool(name="rot", bufs=1))
        tab_pool = ctx.enter_context(tc.tile_pool(name="tab", bufs=1))
        exp_pool = ctx.enter_context(tc.tile_pool(name="exp", bufs=16))
        rc_pool = ctx.enter_context(tc.tile_pool(name="rc", bufs=2))
        osb_pool = ctx.enter_context(tc.tile_pool(name="osb", bufs=2))
        psA = ctx.enter_context(tc.tile_pool(name="psA", bufs=2, space="PSUM"))
        psS = ctx.enter_context(tc.tile_pool(name="psS", bufs=2, space="PSUM"))
        psC = ctx.enter_context(tc.tile_pool(name="psC", bufs=2, space="PSUM"))

        v_t = [v_pool.tile([128, H * (DH + 1)], BF, tag="v", name="v")
               for _ in range(KT_S)]
        ctxT_t = [ctxT_pool.tile([128, SQ], BF, tag="ctxT", name="ctxT")
                  for _ in range(KT_D)]
        for i in range(KT_S):
            # only the per-head ones-columns need initializing; data columns
            # are fully overwritten by the V-projection eviction copies
            vcol = v_t[i][:].rearrange("p (h c) -> p h c", c=DH + 1)
            nc.vector.memset(vcol[:, :, DH:DH + 1], 1.0)

        # batched input loads: one large DMA per tensor (the ~2us fixed cost
        # per dma_start dominates 512KB transfers; 16 ports need one big AP)
        x_all = x_pool.tile([128, KT_D * S], BF, tag="x", name="x_all")
        xq_all = xq_pool.tile([128, KT_D * SQ], BF, tag="xq", name="xq_all")
        x_t = [x_all[:, i * S:(i + 1) * S] for i in range(KT_D)]
        xq_t = [xq_all[:, i * SQ:(i + 1) * SQ] for i in range(KT_D)]
        nc.sync.dma_start(
            x_all[:].rearrange("p (i c) -> p i c", c=S),
            xT.rearrange("(i p) c -> p i c", p=128))
        nc.sync.dma_start(
            xq_all[:].rearrange("p (i c) -> p i c", c=SQ),
            xqT.rearrange("(i p) c -> p i c", p=128))
        cosq_t = tab_pool.tile([128, SQ], BF, tag="cq")
        sinq_t = tab_pool.tile([128, SQ], BF, tag="sq")
        cosk_t = tab_pool.tile([128, S], BF, tag="ck")
        sink_t = tab_pool.tile([128, S], BF, tag="sk")
        nc.sync.dma_start(cosq_t[:], cosq[:])
        nc.sync.dma_start(sinq_t[:], sinrq[:])
        nc.sync.dma_start(cosk_t[:], cosk[:])
        nc.sync.dma_start(sink_t[:], sinrk[:])

        # ---- V projection: v[t, o] = x.T @ wvT, strided into 65-col blocks
        wv_all = wv_pool.tile([128, KT_D * D], BF, tag="w", name="wv_all")
        wv_t = [wv_all[:, i * D:(i + 1) * D] for i in range(KT_D)]
        nc.sync.dma_start(
            wv_all[:].rearrange("p (i c) -> p i c", c=D),
            wvT.rearrange("(i p) c -> p i c", p=128))
        for m in range(KT_S):
            for n in range(NO):
                ps = psA.tile([128, 512], F32, tag="psA")
                for k in range(KT_D):
                    nc.tensor.matmul(
                        ps[:], x_t[k][:, m * 128:(m + 1) * 128],
                        wv_t[k][:, n * 512:(n + 1) * 512],
                        start=(k == 0), stop=(k == KT_D - 1))
                dst = v_t[m][:].rearrange("p (h c) -> p h c", c=DH + 1)
                src = ps[:].rearrange("p (h c) -> p h c", c=DH)
                nc.vector.tensor_copy(
                    dst[:, n * 8:(n + 1) * 8, 0:DH], src[:])

        def load_w_ct(w_dram, ct):
            """One [128, 8*128] tile: col-slice ct of all 8 k-tiles of w."""
            t = wkq_pool.tile([128, D], BF, tag="wkq", name="wkq")
            src = w_dram[:, ct * 128:(ct + 1) * 128].rearrange(
                "(k p) c -> p k c", p=128)
            dst = t[:].rearrange("p (k c) -> p k c", c=128)
            nc.sync.dma_start(dst, src)
            return t

        def project_rope(w_ct, x_tiles, nt, nchunks, cos_t, sin_t, o,
                         o_hi=None):
            """o = rope(w_ct.T @ x) for one 128-row o-tile.

            With o_hi given, the two 64-row halves are written to o[0:64]
            and o_hi[64:128] (split per-head tiles whose other halves stay
            zero, so score matmuls can run full C=128 contraction)."""
            T = nt
            raw = raw_pool.tile([128, S], BF, tag="raw")
            for n in range(nchunks):
                ps = psA.tile([128, 512], F32, tag="psA")
                for k in range(KT_D):
                    nc.tensor.matmul(
                        ps[:], w_ct[:, k * 128:(k + 1) * 128],
                        x_tiles[k][:, n * 512:(n + 1) * 512],
                        start=(k == 0), stop=(k == KT_D - 1))
                nc.scalar.activation(
                    raw[:, n * 512:(n + 1) * 512], ps[:],
                    mybir.ActivationFunctionType.Copy)
            rot = rot_pool.tile([128, S], BF, tag="rot")
            for b0 in (0, 64):
                nc.vector.tensor_copy(rot[b0:b0 + 32, 0:T],
                                      raw[b0 + 32:b0 + 64, 0:T])
                nc.vector.tensor_copy(rot[b0 + 32:b0 + 64, 0:T],
                                      raw[b0:b0 + 32, 0:T])
            halves = ((o, 0), (o if o_hi is None else o_hi, 64))
            nc.vector.tensor_mul(rot[:, 0:T], rot[:, 0:T], sin_t[:, 0:T])
            for dst, b0 in halves:
                sl = slice(b0, b0 + 64)
                nc.vector.tensor_mul(dst[sl, 0:T], raw[sl, 0:T],
                                     cos_t[sl, 0:T])
                nc.vector.tensor_add(dst[sl, 0:T], dst[sl, 0:T],
                                     rot[sl, 0:T])

        for ct in range(NCT):
            # padded per-head K tiles: krA rows 64-127 and krB rows 0-63 are
            # zero, so score matmuls run full C=128 stationaries (FWL on,
            # LDWEIGHTS fast path) instead of the much slower C=64 form
            if pad_scores:
                krA = kr_pool.tile([128, S], BF, tag="krA", name="krA")
                krB = kr_pool.tile([128, S], BF, tag="krB", name="krB")
                if ct < 2:
                    # zero the pad halves once per pool slot; later cts
                    # reuse the slots and nothing rewrites the pad rows
                    nc.gpsimd.memset(krA[64:128, :], 0.0)
                    nc.gpsimd.memset(krB[0:64, :], 0.0)
            else:
                krA = krB = kr_pool.tile([128, S], BF, tag="kr", name="kr")
            qr = qr_pool.tile([128, SQ], BF, tag="qr", name="qr")
            wk_ct = load_w_ct(wkT, ct)
            wq_ct = load_w_ct(wqT, ct)
            project_rope(wk_ct, x_t, S, NS, cosk_t, sink_t, krA,
                         o_hi=krB if pad_scores else None)
            project_rope(wq_ct, xq_t, SQ, NQ, cosq_t, sinq_t, qr)

            # -------- attention for heads (2ct, 2ct+1) --------
            # scores MMs for the head pair are emitted adjacently at
            # partitions 0-63 / 64-127 -> PE row-group packing runs them
            # concurrently.  One 2-bank fp32 PSUM tile per (head, kt); one
            # Exp instruction over the whole [128, 1024].
            expt = [[None] * KT_S for _ in range(2)]
            for kt in range(KT_S):
                # kt-major, half-minor allocation: halves use distinct slots
                for half in range(2):
                    expt[half][kt] = exp_pool.tile([128, SQ], BF, tag="exp",
                                                   name="exp")
            for kt in range(KT_S):
                pss = [psS.tile([128, SQ], F32, tag="psS", name="pss")
                       for _ in range(2)]
                # stationary (padded kr slice, C=128) reused across both
                # q-chunks so the LDWEIGHTS amortizes over 2 matmuls
                for half, krh in ((0, krA), (1, krB)):
                    b0 = half * 64
                    st = (krh[:, kt * 128:(kt + 1) * 128] if pad_scores
                          else krh[b0:b0 + 64, kt * 128:(kt + 1) * 128])
                    for qb in range(NQ):
                        nc.tensor.matmul(
                            pss[half][:, qb * 512:(qb + 1) * 512],
                            st,
                            (qr[:, qb * 512:(qb + 1) * 512] if pad_scores
                             else qr[b0:b0 + 64, qb * 512:(qb + 1) * 512]),
                            start=True, stop=True)
                for half in range(2):
                    nc.scalar.activation(
                        expt[half][kt][:], pss[half][:],
                        mybir.ActivationFunctionType.Exp, scale=0.125)
            for half in range(2):
                h = 2 * ct + half
                b0 = half * 64
                # qb-interleaved PV: each exp tile is consumed by both qb
                # accumulators at kt time, then freed -> exp slots recycle
                # at PV pace instead of per-head granularity.
                cpss = [psC.tile([65, 512], F32, tag="psC", name="cps")
                        for _ in range(NQ)]
                for kt in range(KT_S):
                    for qb in range(NQ):
                        nc.tensor.matmul(
                            cpss[qb][:],
                            v_t[kt][:, h * (DH + 1):(h + 1) * (DH + 1)],
                            expt[half][kt][:, qb * 512:(qb + 1) * 512],
                            start=(kt == 0), stop=(kt == KT_S - 1))
                for qb in range(NQ):
                    cps = cpss[qb]
                    # reciprocal lands at partition 0: the gpsimd broadcast
                    # hardware reads the source on Q7 core 0 (partitions
                    # 0-15) and pushes right, so the source row must live in
                    # partition 0 (the interp is laxer than the hardware).
                    rc = rc_pool.tile([1, 512], BF, tag="rc")
                    with nc.allow_low_precision(reason="bf16 softmax denom"):
                        nc.vector.reciprocal(rc[0:1, :], cps[64:65, :])
                    bcs = rc_pool.tile([64, 512], BF, tag="bcs", name="bcs")
                    nc.gpsimd.partition_broadcast(bcs[:], rc[0:1, :],
                                                  channels=64)
                    nc.vector.tensor_mul(
                        ctxT_t[ct][b0:b0 + 64, qb * 512:(qb + 1) * 512],
                        cps[0:64, :], bcs[:])

        # -------- output projection --------
        wo_all = wv_pool.tile([128, KT_D * D], BF, tag="w", name="wo_all")
        wo_t = [wo_all[:, i * D:(i + 1) * D] for i in range(KT_D)]
        nc.sync.dma_start(
            wo_all[:].rearrange("p (i c) -> p i c", c=D),
            woT.rearrange("(i p) c -> p i c", p=128))
        for m in range(KT_SQ):
            for n in range(NO):
                ps = psA.tile([128, 512], F32, tag="psA")
                for k in range(KT_D):
                    nc.tensor.matmul(
                        ps[:], ctxT_t[k][:, m * 128:(m + 1) * 128],
                        wo_t[k][:, n * 512:(n + 1) * 512],
                        start=(k == 0), stop=(k == KT_D - 1))
                ot = osb_pool.tile([128, 512], F32, tag="osb")
                nc.scalar.activation(ot[:], ps[:],
                                     mybir.ActivationFunctionType.Copy)
                nc.sync.dma_start(
                    out[m * 128:(m + 1) * 128, n * 512:(n + 1) * 512], ot[:])

    nc.compile()
    return nc


_NC = None
LAST_RESULT = None
LAST_IN_MAPS = None


def _get_nc():
    global _NC
    if _NC is None:
        _NC = _build()
    return _NC


def kernel(x, cos, sin, wq, wk, wv, wo):
    global LAST_RESULT, LAST_IN_MAPS
    x = np.asarray(x)
    cos = np.asarray(cos)
    sin = np.asarray(sin)

    def bf(a):
        return np.ascontiguousarray(a, dtype=np.float32).astype(BF_NP)

    cosT = cos[0, :, 0, :].T.astype(np.float32)   # [DH, S]
    sinT = sin[0, :, 0, :].T.astype(np.float32)
    sinr = np.concatenate([-sinT[:DH // 2], sinT[DH // 2:]], axis=0)
    cos2 = np.concatenate([cosT, cosT], axis=0)   # [128, S]
    sinr2 = np.concatenate([sinr, sinr], axis=0)

    wqT, wkT, wvT, woT = (bf(w.T) for w in (wq, wk, wv, wo))
    in_maps = []
    for c in range(NCORES):
        b, half = c // 2, c % 2
        q0 = half * SQ
        xTb = bf(x[b].T)
        in_maps.append({
            "xT": xTb,
            "xqT": np.ascontiguousarray(xTb[:, q0:q0 + SQ]),
            "wqT": wqT, "wkT": wkT, "wvT": wvT, "woT": woT,
            "cosq": bf(cos2[:, q0:q0 + SQ]),
            "sinrq": bf(sinr2[:, q0:q0 + SQ]),
            "cosk": bf(cos2), "sinrk": bf(sinr2),
        })

    LAST_IN_MAPS = in_maps
    nc = _get_nc()
    res = bass_utils.run_bass_kernel_spmd(nc, in_maps,
                                          core_ids=list(range(NCORES)))
    LAST_RESULT = res
    out_full = np.empty((B, S, D), np.float32)
    for c in range(NCORES):
        b, half = c // 2, c % 2
        out_full[b, half * SQ:(half + 1) * SQ, :] = res.results[c]["out"]
    return out_full


# revision 8
# speedup vs baseline: 1.0993x; 1.0993x over previous
"""Distributed multi-head attention (RoPE + SDPA + out-proj) for one TRN2 chip.

Sharding: 8 cores = 4 batches x 2 query-halves (zero collectives).  v2
restructure vs baseline:
  - projections for o-tile ct (a head pair) interleave with attention of the
    same pair, so ACT exp work overlaps PE projection work
  - scores: one N=1024 bf16-PSUM matmul per (head, kt) (no accumulation, so
    bf16 PSUM out is exact-enough); head pairs sit at partitions 0-63/64-127,
    emitted adjacently -> PE row-group packing runs them concurrently
  - exp: one ACT instruction per (head, kt) over [128, 1024]
  - softmax-denominator broadcast copy moved ACT -> DVE
  - xq kept as input (per-core query-half offset cannot be in SPMD code)
"""

from contextlib import ExitStack, nullcontext

import ml_dtypes
import numpy as np

import concourse.bass as bass
import concourse.tile as tile
from concourse import bacc, bass_utils, mybir

B, S, D, H = 4, 2048, 1024, 16
DH = D // H
SQ = S // 2          # queries per core
NCORES = 8
BF = mybir.dt.bfloat16
F32 = mybir.dt.float32
BF_NP = ml_dtypes.bfloat16

KT_D = D // 128      # 8  d-tiles (contraction for projections)
KT_S = S // 128      # 16 seq k-tiles
KT_SQ = SQ // 128    # 8  out t-tiles
NQ = SQ // 512       # 2  q chunks (512-wide, fp32-psum granularity)
NS = S // 512        # 4  seq chunks
NO = D // 512        # 2  o chunks
NCT = KT_D           # 8  head-pair tiles


def _build(reps=1, pad_scores=True):
    nc = bacc.Bacc("TRN2", target_bir_lowering=False, debug=False,
                   num_devices=NCORES)

    xT = nc.dram_tensor("xT", [D, S], BF, kind="ExternalInput").ap()
    xqT = nc.dram_tensor("xqT", [D, SQ], BF, kind="ExternalInput").ap()
    wqT = nc.dram_tensor("wqT", [D, D], BF, kind="ExternalInput").ap()
    wkT = nc.dram_tensor("wkT", [D, D], BF, kind="ExternalInput").ap()
    wvT = nc.dram_tensor("wvT", [D, D], BF, kind="ExternalInput").ap()
    woT = nc.dram_tensor("woT", [D, D], BF, kind="ExternalInput").ap()
    cosq = nc.dram_tensor("cosq", [128, SQ], BF, kind="ExternalInput").ap()
    sinrq = nc.dram_tensor("sinrq", [128, SQ], BF, kind="ExternalInput").ap()
    cosk = nc.dram_tensor("cosk", [128, S], BF, kind="ExternalInput").ap()
    sinrk = nc.dram_tensor("sinrk", [128, S], BF, kind="ExternalInput").ap()
    out = nc.dram_tensor("out", [SQ, D], F32, kind="ExternalOutput").ap()

    with tile.TileContext(nc) as tc, \
         (tc.For_i(0, reps) if reps > 1 else nullcontext()), \
         ExitStack() as ctx:
        qr_pool = ctx.enter_context(tc.tile_pool(name="qr", bufs=2))
        kr_pool = ctx.enter_context(tc.tile_pool(name="kr", bufs=2))
        v_pool = ctx.enter_context(tc.tile_pool(name="v", bufs=KT_S))
        ctxT_pool = ctx.enter_context(tc.tile_pool(name="ctxT", bufs=KT_D))
        x_pool = ctx.enter_context(tc.tile_pool(name="x", bufs=1))
        xq_pool = ctx.enter_context(tc.tile_pool(name="xq", bufs=1))
        wv_pool = ctx.enter_context(tc.tile_pool(name="wv", bufs=1))
        wkq_pool = ctx.enter_context(tc.tile_pool(name="wkq", bufs=4))
        raw_pool = ctx.enter_context(tc.tile_pool(name="raw", bufs=2))
        rot_pool = ctx.enter_context(tc.tile_pool(name="rot", bufs=1))
        tab_pool = ctx.enter_context(tc.tile_pool(name="tab", bufs=1))
        exp_pool = ctx.enter_context(tc.tile_pool(name="exp", bufs=16))
        rc_pool = ctx.enter_context(tc.tile_pool(name="rc", bufs=2))
        osb_pool = ctx.enter_context(tc.tile_pool(name="osb", bufs=2))
        psA = ctx.enter_context(tc.tile_pool(name="psA", bufs=2, space="PSUM"))
        psS = ctx.enter_context(tc.tile_pool(name="psS", bufs=2, space="PSUM"))
        psC = ctx.enter_context(tc.tile_pool(name="psC", bufs=2, space="PSUM"))

        v_t = [v_pool.tile([128, H * (DH + 1)], BF, tag="v", name="v")
               for _ in range(KT_S)]
        ctxT_t = [ctxT_pool.tile([128, SQ], BF, tag="ctxT", name="ctxT")
                  for _ in range(KT_D)]
        for i in range(KT_S):
            # only the per-head ones-columns need initializing; data columns
            # are fully overwritten by the V-projection eviction copies
            vcol = v_t[i][:].rearrange("p (h c) -> p h c", c=DH + 1)
            nc.vector.memset(vcol[:, :, DH:DH + 1], 1.0)

        # batched input loads: one large DMA per tensor (the ~2us fixed cost
        # per dma_start dominates 512KB transfers; 16 ports need one big AP)
        x_all = x_pool.tile([128, KT_D * S], BF, tag="x", name="x_all")
        xq_all = xq_pool.tile([128, KT_D * SQ], BF, tag="xq", name="xq_all")
        x_t = [x_all[:, i * S:(i + 1) * S] for i in range(KT_D)]
        xq_t = [xq_all[:, i * SQ:(i + 1) * SQ] for i in range(KT_D)]
        nc.sync.dma_start(
            x_all[:].rearrange("p (i c) -> p i c", c=S),
            xT.rearrange("(i p) c -> p i c", p=128))
        nc.sync.dma_start(
            xq_all[:].rearrange("p (i c) -> p i c", c=SQ),
            xqT.rearrange("(i p) c -> p i c", p=128))
        cosq_t = tab_pool.tile([128, SQ], BF, tag="cq")
        sinq_t = tab_pool.tile([128, SQ], BF, tag="sq")
        cosk_t = tab_pool.tile([128, S], BF, tag="ck")
        sink_t = tab_pool.tile([128, S], BF, tag="sk")
        nc.sync.dma_start(cosq_t[:], cosq[:])
        nc.sync.dma_start(sinq_t[:], sinrq[:])
        nc.sync.dma_start(cosk_t[:], cosk[:])
        nc.sync.dma_start(sink_t[:], sinrk[:])

        # ---- V projection: v[t, o] = x.T @ wvT, strided into 65-col blocks
        wv_all = wv_pool.tile([128, KT_D * D], BF, tag="w", name="wv_all")
        wv_t = [wv_all[:, i * D:(i + 1) * D] for i in range(KT_D)]
        nc.sync.dma_start(
            wv_all[:].rearrange("p (i c) -> p i c", c=D),
            wvT.rearrange("(i p) c -> p i c", p=128))
        for m in range(KT_S):
            ps = psS.tile([128, 2 * 512], F32, tag="psS", name="ps")
            for k in range(KT_D):
                # stationary (x slice) reused across both o-chunks
                for n in range(NO):
                    nc.tensor.matmul(
                        ps[:, n * 512:(n + 1) * 512],
                        x_t[k][:, m * 128:(m + 1) * 128],
                        wv_t[k][:, n * 512:(n + 1) * 512],
                        start=(k == 0), stop=(k == KT_D - 1))
            dst = v_t[m][:].rearrange("p (h c) -> p h c", c=DH + 1)
            src = ps[:].rearrange("p (h c) -> p h c", c=DH)
            nc.vector.tensor_copy(dst[:, :, 0:DH], src[:])

        def load_w_ct(w_dram, ct):
            """One [128, 8*128] tile: col-slice ct of all 8 k-tiles of w."""
            t = wkq_pool.tile([128, D], BF, tag="wkq", name="wkq")
            src = w_dram[:, ct * 128:(ct + 1) * 128].rearrange(
                "(k p) c -> p k c", p=128)
            dst = t[:].rearrange("p (k c) -> p k c", c=128)
            nc.sync.dma_start(dst, src)
            return t

        def project_rope(w_ct, x_tiles, nt, nchunks, cos_t, sin_t, o,
                         o_hi=None):
            """o = rope(w_ct.T @ x) for one 128-row o-tile.

            With o_hi given, the two 64-row halves are written to o[0:64]
            and o_hi[64:128] (split per-head tiles whose other halves stay
            zero, so score matmuls can run full C=128 contraction)."""
            T = nt
            raw = raw_pool.tile([128, S], BF, tag="raw")
            for n in range(nchunks):
                ps = psA.tile([128, 512], F32, tag="psA")
                for k in range(KT_D):
                    nc.tensor.matmul(
                        ps[:], w_ct[:, k * 128:(k + 1) * 128],
                        x_tiles[k][:, n * 512:(n + 1) * 512],
                        start=(k == 0), stop=(k == KT_D - 1))
                nc.scalar.activation(
                    raw[:, n * 512:(n + 1) * 512], ps[:],
                    mybir.ActivationFunctionType.Copy)
            rot = rot_pool.tile([128, S], BF, tag="rot")
            for b0 in (0, 64):
                nc.vector.tensor_copy(rot[b0:b0 + 32, 0:T],
                                      raw[b0 + 32:b0 + 64, 0:T])
                nc.vector.tensor_copy(rot[b0 + 32:b0 + 64, 0:T],
                                      raw[b0:b0 + 32, 0:T])
            halves = ((o, 0), (o if o_hi is None else o_hi, 64))
            nc.vector.tensor_mul(rot[:, 0:T], rot[:, 0:T], sin_t[:, 0:T])
            for dst, b0 in halves:
                sl = slice(b0, b0 + 64)
                nc.vector.tensor_mul(dst[sl, 0:T], raw[sl, 0:T],
                                     cos_t[sl, 0:T])
                nc.vector.tensor_add(dst[sl, 0:T], dst[sl, 0:T],
                                     rot[sl, 0:T])

        for ct in range(NCT):
            # padded per-head K tiles: krA rows 64-127 and krB rows 0-63 are
            # zero, so score matmuls run full C=128 stationaries (FWL on,
            # LDWEIGHTS fast path) instead of the much slower C=64 form
            if pad_scores:
                krA = kr_pool.tile([128, S], BF, tag="krA", name="krA")
                krB = kr_pool.tile([128, S], BF, tag="krB", name="krB")
                if ct < 2:
                    # zero the pad halves once per pool slot; later cts
                    # reuse the slots and nothing rewrites the pad rows
                    nc.gpsimd.memset(krA[64:128, :], 0.0)
                    nc.gpsimd.memset(krB[0:64, :], 0.0)
            else:
                krA = krB = kr_pool.tile([128, S], BF, tag="kr", name="kr")
            qr = qr_pool.tile([128, SQ], BF, tag="qr", name="qr")
            wk_ct = load_w_ct(wkT, ct)
            wq_ct = load_w_ct(wqT, ct)
            project_rope(wk_ct, x_t, S, NS, cosk_t, sink_t, krA,
                         o_hi=krB if pad_scores else None)
            project_rope(wq_ct, xq_t, SQ, NQ, cosq_t, sinq_t, qr)

            # -------- attention for heads (2ct, 2ct+1) --------
            # scores MMs for the head pair are emitted adjacently at
            # partitions 0-63 / 64-127 -> PE row-group packing runs them
            # concurrently.  One 2-bank fp32 PSUM tile per (head, kt); one
            # Exp instruction over the whole [128, 1024].
            expt = [[None] * KT_S for _ in range(2)]
            for kt in range(KT_S):
                # kt-major, half-minor allocation: halves use distinct slots
                for half in range(2):
                    expt[half][kt] = exp_pool.tile([128, SQ], BF, tag="exp",
                                                   name="exp")
            for kt in range(KT_S):
                pss = [psS.tile([128, SQ], F32, tag="psS", name="pss")
                       for _ in range(2)]
                # stationary (padded kr slice, C=128) reused across both
                # q-chunks so the LDWEIGHTS amortizes over 2 matmuls
                for half, krh in ((0, krA), (1, krB)):
                    b0 = half * 64
                    st = (krh[:, kt * 128:(kt + 1) * 128] if pad_scores
                          else krh[b0:b0 + 64, kt * 128:(kt + 1) * 128])
                    for qb in range(NQ):
                        nc.tensor.matmul(
                            pss[half][:, qb * 512:(qb + 1) * 512],
                            st,
                            (qr[:, qb * 512:(qb + 1) * 512] if pad_scores
                             else qr[b0:b0 + 64, qb * 512:(qb + 1) * 512]),
                            start=True, stop=True)
                for half in range(2):
                    nc.scalar.activation(
                        expt[half][kt][:], pss[half][:],
                        mybir.ActivationFunctionType.Exp, scale=0.125)
            for half in range(2):
                h = 2 * ct + half
                b0 = half * 64
                # qb-interleaved PV: each exp tile is consumed by both qb
                # accumulators at kt time, then freed -> exp slots recycle
                # at PV pace instead of per-head granularity.
                cpss = [psC.tile([65, 512], F32, tag="psC", name="cps")
                        for _ in range(NQ)]
                for kt in range(KT_S):
                    for qb in range(NQ):
                        nc.tensor.matmul(
                            cpss[qb][:],
                            v_t[kt][:, h * (DH + 1):(h + 1) * (DH + 1)],
                            expt[half][kt][:, qb * 512:(qb + 1) * 512],
                            start=(kt == 0), stop=(kt == KT_S - 1))
                for qb in range(NQ):
                    cps = cpss[qb]
                    # reciprocal lands at partition 0: the gpsimd broadcast
                    # hardware reads the source on Q7 core 0 (partitions
                    # 0-15) and pushes right, so the source row must live in
                    # partition 0 (the interp is laxer than the hardware).
                    rc = rc_pool.tile([1, 512], BF, tag="rc")
                    with nc.allow_low_precision(reason="bf16 softmax denom"):
                        nc.vector.reciprocal(rc[0:1, :], cps[64:65, :])
                    bcs = rc_pool.tile([64, 512], BF, tag="bcs", name="bcs")
                    nc.gpsimd.partition_broadcast(bcs[:], rc[0:1, :],
                                                  channels=64)
                    nc.vector.tensor_mul(
                        ctxT_t[ct][b0:b0 + 64, qb * 512:(qb + 1) * 512],
                        cps[0:64, :], bcs[:])

        # -------- output projection --------
        wo_all = wv_pool.tile([128, KT_D * D], BF, tag="w", name="wo_all")
        wo_t = [wo_all[:, i * D:(i + 1) * D] for i in range(KT_D)]
        nc.sync.dma_start(
            wo_all[:].rearrange("p (i c) -> p i c", c=D),
            woT.rearrange("(i p) c -> p i c", p=128))
        for m in range(KT_SQ):
            for n in range(NO):
                ps = psA.tile([128, 512], F32, tag="psA")
                for k in range(KT_D):
                    nc.tensor.matmul(
                        ps[:], ctxT_t[k][:, m * 128:(m + 1) * 128],
                        wo_t[k][:, n * 512:(n + 1) * 512],
                        start=(k == 0), stop=(k == KT_D - 1))
                ot = osb_pool.tile([128, 512], F32, tag="osb")
                nc.scalar.activation(ot[:], ps[:],
                                     mybir.ActivationFunctionType.Copy)
                nc.sync.dma_start(
                    out[m * 128:(m + 1) * 128, n * 512:(n + 1) * 512], ot[:])

    nc.compile()
    return nc


_NC = None
LAST_RESULT = None
LAST_IN_MAPS = None


def _get_nc():
    global _NC
    if _NC is None:
        _NC = _build()
    return _NC


def kernel(x, cos, sin, wq, wk, wv, wo):
    global LAST_RESULT, LAST_IN_MAPS
    x = np.asarray(x)
    cos = np.asarray(cos)
    sin = np.asarray(sin)

    def bf(a):
        return np.ascontiguousarray(a, dtype=np.float32).astype(BF_NP)

    cosT = cos[0, :, 0, :].T.astype(np.float32)   # [DH, S]
    sinT = sin[0, :, 0, :].T.astype(np.float32)
    sinr = np.concatenate([-sinT[:DH // 2], sinT[DH // 2:]], axis=0)
    cos2 = np.concatenate([cosT, cosT], axis=0)   # [128, S]
    sinr2 = np.concatenate([sinr, sinr], axis=0)

    wqT, wkT, wvT, woT = (bf(w.T) for w in (wq, wk, wv, wo))
    in_maps = []
    for c in range(NCORES):
        b, half = c // 2, c % 2
        q0 = half * SQ
        xTb = bf(x[b].T)
        in_maps.append({
            "xT": xTb,
            "xqT": np.ascontiguousarray(xTb[:, q0:q0 + SQ]),
            "wqT": wqT, "wkT": wkT, "wvT": wvT, "woT": woT,
            "cosq": bf(cos2[:, q0:q0 + SQ]),
            "sinrq": bf(sinr2[:, q0:q0 + SQ]),
            "cosk": bf(cos2), "sinrk": bf(sinr2),
        })

    LAST_IN_MAPS = in_maps
    nc = _get_nc()
    res = bass_utils.run_bass_kernel_spmd(nc, in_maps,
                                          core_ids=list(range(NCORES)))
    LAST_RESULT = res
    out_full = np.empty((B, S, D), np.float32)
    for c in range(NCORES):
        b, half = c // 2, c % 2
        out_full[b, half * SQ:(half + 1) * SQ, :] = res.results[c]["out"]
    return out_full
